# revision 10
# baseline (speedup 1.0000x reference)
"""Trainium2 Bass kernel for nn_AttentionSparseMax (v3).

Computation (see the reference model):
  q/k/v projections -> 16-head attention scores -> sparsemax per row ->
  attn @ v -> Wo projection -> concat(enc, out) -> relu MLP -> classifier.

Sharding across 8 NeuronCores (SPMD, per-core weight views):
  - Attention head-sharded (2 heads/core); MLP data-parallel on the
    core's 256 queries with bf16 weights AllGather'd from per-core
    transposed slices; Wo partials combined with per-512-query-block
    ReduceScatters (bf16).

v3 changes vs v2 (baseline 1.37ms):
  - All-bf16 attention datapath (scores, attn, v, all matmul
    stationaries).  bf16 stationaries get fast weight loads; bf16
    matmuls run 1 cyc/col instead of the f32r 2x penalty observed in
    the v2 trace (LDWEIGHTS 286ns + MATMUL 794ns per 512-col MM).
  - tau is fused into the pass-B score matmul as two extra contraction
    rows (K=66): stationary rows 64/65 are ones, moving rows carry
    -tau_hi/-tau_lo (bf16 hi+lo keeps tau at ~f32 precision).  The
    PSUM tile then holds S^T - tau directly, so the eviction is a plain
    scalar-engine Relu(copy) -> the vector engine is freed for
    MAX8/Newton (it was 100% busy and the pipeline critical path), and
    the rank-1 -tau*colsum(v) correction disappears.
  - AV matmuls col-tiled (head0 -> PSUM partitions 0-63, head1 64-127)
    with per-element has_written accumulation instead of zero-padded
    full-M stationaries: half the AV cycles.
  - Per-nb (512-query block) software pipelining: pass A (scores +
    DVE max8 candidates) of block nb+1 overlaps pass B + AV + Wo of
    block nb; Newton tau (4 iters) runs per-nb between them.  Pass A
    of block 0 is interleaved into the memory_set projection loop so
    the DVE starts ~45us into the kernel.
  - MLP h and proj->finT transposes via the DMA XBAR bf16 transpose
    instead of PE transpose + scalar eviction.
  - enc/mem transposes on the PE in f32r (1.5 cyc/row vs 2.0 for f32).
"""

import numpy as np

import concourse.bass as bass
import concourse.mybir as mybir
from concourse import bacc
from concourse.tile import TileContext
from concourse.bass_utils import run_bass_kernel_spmd
from concourse.masks import make_identity

dt = mybir.dt
F32 = dt.float32
F32R = dt.float32r
BF16 = dt.bfloat16
AF = mybir.ActivationFunctionType
OP = mybir.AluOpType
AX = mybir.AxisListType

N, M, D, OUT = 2048, 4096, 1024, 1000
H, DH = 16, 64
NCORES = 8
HPC = H // NCORES          # heads per core (2)
DH2 = HPC * DH             # 128
ISL = (4 * D) // NCORES    # 512 hidden units per core's W1/W2 slice
NL = N // NCORES           # 256 queries per core (4 groups of 64)
SCALE = 1.0 / float(np.sqrt(np.float32(D)))

NEWTON_ITERS = 4
KC = (M // 256) * 8        # 128 candidates per row (top-8 per 256-chunk)
KA = DH + 2                # 66: contraction rows of pass B (k rows + 2 tau)
OC = OUT // 2              # 500-wide classifier chunks


def build_kernel() -> bacc.Bacc:
    nc = bacc.Bacc("TRN2", target_bir_lowering=False, debug=False,
                   num_devices=NCORES)

    enc = nc.dram_tensor("encoder_output", [N, D], F32R,
                         kind="ExternalInput").ap()
    encl = nc.dram_tensor("enc_local", [NL, D], F32R,
                          kind="ExternalInput").ap()
    mem = nc.dram_tensor("memory_set", [M, D], F32R,
                         kind="ExternalInput").ap()
    Wq = nc.dram_tensor("Wq", [D, D], F32R, kind="ExternalInput").ap()
    Wk = nc.dram_tensor("Wk", [D, D], F32R, kind="ExternalInput").ap()
    Wv = nc.dram_tensor("Wv", [D, D], F32R, kind="ExternalInput").ap()
    Wo = nc.dram_tensor("Wo", [D, D], F32R, kind="ExternalInput").ap()
    W1 = nc.dram_tensor("W1", [4 * D, 2 * D], F32R, kind="ExternalInput").ap()
    W2 = nc.dram_tensor("W2", [OUT, 4 * D], F32R, kind="ExternalInput").ap()
    y = nc.dram_tensor("y", [NL, OUT], F32, kind="ExternalOutput").ap()

    tau_dram = nc.dram_tensor("tau_dram", [HPC, 2, 2, 8, 128], BF16).ap()
    proj_part = nc.dram_tensor("proj_part", [N, D], BF16).ap()
    proj_loc = nc.dram_tensor("proj_loc", [4, N // 4 // NCORES, D], BF16).ap()
    w1t_slice = nc.dram_tensor("w1t_slice", [2 * D, ISL], BF16).ap()
    w1t_all = nc.dram_tensor("w1t_all", [NCORES * 2 * D, ISL], BF16,
                             addr_space="Shared").ap()
    w2t_slice = nc.dram_tensor("w2t_slice", [ISL, OUT], BF16).ap()
    w2t_all = nc.dram_tensor("w2t_all", [4 * D, OUT], BF16,
                             addr_space="Shared").ap()
    rg = [list(range(NCORES))]

    with TileContext(nc) as tc:
        glob_ctx = tc.tile_pool(name="glob", bufs=1)
        glob_pool = glob_ctx.__enter__()
        identf = glob_pool.tile([128, 128], F32, tag="identf")
        identr = glob_pool.tile([128, 128], F32R, tag="identr")
        identb = glob_pool.tile([128, 128], BF16, tag="identb")
        make_identity(nc, identf[:])
        nc.scalar.copy(identr[:], identf[:])
        nc.scalar.copy(identb[:], identf[:])

        with tc.tile_pool(name="per", bufs=1) as per:
            q2 = per.tile([128, N], BF16, tag="q2")       # scaled q^T, 2 heads
            k2 = per.tile([128, M], BF16, tag="k2")       # k^T, 2 heads
            # pass-B operands: rows 0-63 = head h data, rows 64/65 carry
            # the tau fusion (ones on the k side, -tau_hi/-tau_lo on q)
            q2x = [per.tile([KA, N], BF16, tag=f"q2x{h}", name=f"q2x{h}")
                   for h in range(HPC)]
            k2x = [per.tile([KA, M], BF16, tag=f"k2x{h}", name=f"k2x{h}")
                   for h in range(HPC)]
            # v in [m, dh] layout per head (AV stationaries, col-tiled)
            v2 = [per.tile([128, 32, DH], BF16, tag=f"v2{h}", name=f"v2{h}")
                  for h in range(HPC)]
            wqT = per.tile([128, 8, 128], BF16, tag="wqT")
            wkT = per.tile([128, 8, 128], BF16, tag="wkT")
            wvT = per.tile([128, 8, 128], BF16, tag="wvT")
            woT = per.tile([DH2, D], BF16, tag="woT")
            finT = per.tile([128, 16, NL], BF16, tag="finT")
            hT = per.tile([128, 32, NL], BF16, tag="hT")
            cands = [per.tile([128, 16, KC], BF16, tag=f"cand{h}",
                              name=f"cand{h}") for h in range(HPC)]

            for h in range(HPC):
                nc.vector.memset(k2x[h][DH:KA, :], 1.0)

            # ============ phase W-A: Wq/Wk/Wv/Wo transposes ===============
            with (
                tc.tile_pool(name="stw", bufs=2) as stw,
                tc.tile_pool(name="psw", bufs=2, space="PSUM") as psw,
            ):
                for w_dram, w_tile in ((Wq, wqT), (Wk, wkT), (Wv, wvT)):
                    wn = stw.tile([128, D], F32R, tag="w_nat", name="w_nat")
                    nc.scalar.dma_start(wn[:], w_dram[0:DH2, :])
                    for half in range(2):
                        pt = psw.tile([128, 512], F32R, tag="ps_wt",
                                      name="ps_wt")
                        for s in range(4):
                            i = half * 4 + s
                            nc.tensor.transpose(
                                pt[:, s * 128:(s + 1) * 128],
                                wn[:, i * 128:(i + 1) * 128], identr[:])
                        for s in range(4):
                            i = half * 4 + s
                            nc.scalar.copy(w_tile[:, i, :],
                                           pt[:, s * 128:(s + 1) * 128])
                for jt in range(8):
                    won = stw.tile([128, 128], F32R, tag="wo_nat",
                                   name="wo_nat")
                    nc.scalar.dma_start(won[:],
                                        Wo[jt * 128:(jt + 1) * 128, 0:DH2])
                    pt = psw.tile([128, 512], F32R, tag="ps_wt", name="ps_wo")
                    nc.tensor.transpose(pt[:, 0:128], won[:], identr[:])
                    nc.scalar.copy(woT[:, jt * 128:(jt + 1) * 128],
                                   pt[:, 0:128])

            # ============ attention-lifetime PSUM pools ====================
            with (
                tc.tile_pool(name="psa", bufs=1, space="PSUM") as psa,
                tc.tile_pool(name="stn", bufs=2) as stn,
            ):

                def pass_a(nb, mb):
                    """Scores for 4 query tiles of block nb against memory
                    chunk mb (row-tiled concurrent head pair) + DVE max8."""
                    for nt in range(4):
                        g = nb * 4 + nt
                        pspair = []
                        for h in range(HPC):
                            r0, r1 = h * DH, (h + 1) * DH
                            psA = psa.tile([128, 512], F32, tag=f"a{h}",
                                           name=f"ps_a{h}")
                            nc.tensor.matmul(
                                psA[:],
                                q2[r0:r1, g * 128:(g + 1) * 128],
                                k2[r0:r1, mb * 512:(mb + 1) * 512],
                                start=True, stop=True)
                            pspair.append(psA)
                        for h in range(HPC):
                            for ch in range(2):
                                k0 = mb * 16 + ch * 8
                                nc.vector.max(
                                    cands[h][:, g, k0:k0 + 8],
                                    pspair[h][:, ch * 256:(ch + 1) * 256])

                def newton_pair(p):
                    """tau for blocks 2p/2p+1 via Newton on the candidate
                    set; writes -tau (bf16 hi+lo rows) into q2x via DRAM."""
                    for h in range(HPC):
                        c3 = cands[h][:, p * 8:(p + 1) * 8, :]
                        mx = stn.tile([128, 8], F32, tag=f"nw_mx{h}")
                        sval = stn.tile([128, 8], F32, tag=f"nw_s{h}")
                        nab = stn.tile([128, 8], F32, tag=f"nw_n{h}")
                        fval = stn.tile([128, 8], F32, tag=f"nw_f{h}")
                        tcur = stn.tile([128, 8], F32, tag=f"nw_t{h}")
                        tb16 = stn.tile([128, 8], BF16, tag=f"nw_tb{h}")
                        tmp3 = stn.tile([128, 8, KC], BF16, tag=f"nw_tmp{h}")
                        ind3 = stn.tile([128, 8, KC], BF16, tag=f"nw_ind{h}")
                        nc.vector.tensor_reduce(mx[:], c3, axis=AX.X,
                                                op=OP.max)
                        nc.vector.tensor_scalar_add(tcur[:], mx[:], -1.0)
                        for it in range(NEWTON_ITERS):
                            nc.vector.tensor_copy(tb16[:], tcur[:])
                            tb = tb16[:].unsqueeze(2).to_broadcast(
                                [128, 8, KC])
                            nc.vector.tensor_tensor(tmp3[:], c3, tb,
                                                    op=OP.max)
                            nc.vector.tensor_tensor(ind3[:], c3, tb,
                                                    op=OP.is_gt)
                            nc.vector.tensor_reduce(sval[:], tmp3[:],
                                                    axis=AX.X, op=OP.add)
                            nc.vector.tensor_reduce(nab[:], ind3[:],
                                                    axis=AX.X, op=OP.add)
                            nc.vector.scalar_tensor_tensor(
                                fval[:], tcur[:], float(-KC), sval[:],
                                op0=OP.mult, op1=OP.add)
                            nc.vector.tensor_scalar_add(fval[:], fval[:],
                                                        -1.0)
                            nc.vector.tensor_scalar_max(nab[:], nab[:], 1.0)
                            nc.vector.reciprocal(nab[:], nab[:])
                            nc.vector.tensor_tensor(fval[:], fval[:], nab[:],
                                                    op=OP.mult)
                            nc.vector.tensor_tensor(tcur[:], tcur[:],
                                                    fval[:], op=OP.add)
                        neg = stn.tile([128, 8], F32, tag=f"nw_neg{h}")
                        thi = stn.tile([128, 8], BF16, tag=f"nw_hi{h}")
                        tlo32 = stn.tile([128, 8], F32, tag=f"nw_lo32{h}")
                        tlo = stn.tile([128, 8], BF16, tag=f"nw_lo{h}")
                        nc.scalar.mul(neg[:], tcur[:], -1.0)
                        nc.scalar.copy(thi[:], neg[:])
                        nc.vector.tensor_tensor(tlo32[:], neg[:], thi[:],
                                                op=OP.subtract)
                        nc.scalar.copy(tlo[:], tlo32[:])
                        for lvl, t in ((0, thi), (1, tlo)):
                            nc.sync.dma_start(
                                tau_dram[h, p, lvl].rearrange("a b -> b a"),
                                t[:])
                            nc.sync.dma_start(
                                q2x[h][DH + lvl:DH + lvl + 1,
                                       p * 1024:(p + 1) * 1024],
                                tau_dram[h, p, lvl]
                                .rearrange("a b -> (a b)").unsqueeze(0))

                # ============ projections ================================
                with (
                    tc.tile_pool(name="st1", bufs=2) as st1,
                    tc.tile_pool(name="tp1", bufs=2) as tp1,
                    tc.tile_pool(name="pstr", bufs=2, space="PSUM") as pstr,
                    tc.tile_pool(name="psqk", bufs=2, space="PSUM") as psqk,
                    tc.tile_pool(name="psv", bufs=2, space="PSUM") as psv,
                ):
                    def enc_block(nb):
                        ens = []
                        for s in range(4):
                            en = st1.tile([128, D], F32R, tag=f"nat{s}",
                                          name="e_nat")
                            nc.sync.dma_start(
                                en[:], enc[nb * 512 + s * 128:
                                           nb * 512 + (s + 1) * 128, :])
                            ens.append(en)
                        etn = tp1.tile([128, 8, 512], BF16, tag="etn")
                        for i in range(8):
                            pt = pstr.tile([128, 512], F32R, tag="ps_tr",
                                           name="ps_tr")
                            for s in range(4):
                                nc.tensor.transpose(
                                    pt[:, s * 128:(s + 1) * 128],
                                    ens[s][:, i * 128:(i + 1) * 128],
                                    identr[:])
                            nc.scalar.copy(etn[:, i, :], pt[:])
                        ps = psqk.tile([128, 512], F32, tag="ps_qk",
                                       name="ps_q")
                        for i in range(8):
                            nc.tensor.matmul(ps[:], wqT[:, i, :],
                                             etn[:, i, :],
                                             start=(i == 0), stop=(i == 7))
                        nc.scalar.mul(q2[:, nb * 512:(nb + 1) * 512], ps[:],
                                      SCALE)
                        nc.scalar.copy(
                            q2x[0][0:DH, nb * 512:(nb + 1) * 512],
                            q2[0:DH, nb * 512:(nb + 1) * 512])
                        nc.sync.dma_start(
                            q2x[1][0:DH, nb * 512:(nb + 1) * 512],
                            q2[DH:DH2, nb * 512:(nb + 1) * 512])

                    for nbb in range(4):
                        enc_block(nbb)

                    # enc_local -> finT rows 0..1023
                    els = []
                    for s in range(2):
                        el = st1.tile([128, D], F32R, tag=f"nat{s}",
                                      name="el")
                        nc.sync.dma_start(el[:],
                                          encl[s * 128:(s + 1) * 128, :])
                        els.append(el)
                    for i in range(8):
                        pt = pstr.tile([128, 512], F32R, tag="ps_tr",
                                       name="ps_trl")
                        for s in range(2):
                            nc.tensor.transpose(
                                pt[:, s * 128:(s + 1) * 128],
                                els[s][:, i * 128:(i + 1) * 128], identr[:])
                        nc.scalar.copy(finT[:, i, :], pt[:, 0:256])

                    # mem -> k2, v2  (+ pass A of blocks 0/1 interleaved)
                    for mb in range(8):
                        mns = []
                        for s in range(4):
                            mn = st1.tile([128, D], F32R, tag=f"nat{s}",
                                          name="m_nat")
                            nc.sync.dma_start(
                                mn[:], mem[mb * 512 + s * 128:
                                           mb * 512 + (s + 1) * 128, :])
                            mns.append(mn)
                        mtn = tp1.tile([128, 8, 512], BF16, tag="etn")
                        for i in range(8):
                            pt = pstr.tile([128, 512], F32R, tag="ps_tr",
                                           name="ps_trm")
                            for s in range(4):
                                nc.tensor.transpose(
                                    pt[:, s * 128:(s + 1) * 128],
                                    mns[s][:, i * 128:(i + 1) * 128],
                                    identr[:])
                            nc.scalar.copy(mtn[:, i, :], pt[:])
                        psk = psqk.tile([128, 512], F32, tag="ps_qk",
                                        name="ps_k")
                        for i in range(8):
                            nc.tensor.matmul(psk[:], wkT[:, i, :],
                                             mtn[:, i, :],
                                             start=(i == 0), stop=(i == 7))
                        nc.scalar.copy(k2[:, mb * 512:(mb + 1) * 512],
                                       psk[:])
                        for sub in range(4):
                            psv_t = psv.tile([128, DH2], F32, tag="ps_v",
                                             name="ps_v")
                            for i in range(8):
                                nc.tensor.matmul(
                                    psv_t[:],
                                    mtn[:, i, sub * 128:(sub + 1) * 128],
                                    wvT[:, i, :],
                                    start=(i == 0), stop=(i == 7))
                            mt = mb * 4 + sub
                            nc.scalar.copy(v2[0][:, mt, :], psv_t[:, 0:DH])
                            nc.scalar.copy(v2[1][:, mt, :],
                                           psv_t[:, DH:DH2])
                        pass_a(0, mb)
                        pass_a(1, mb)

                    # pass-B stationaries (head1 repack shifts partitions
                    # 64-127 -> 0-63, which only the DMA path can do)
                    nc.scalar.copy(k2x[0][0:DH, :], k2[0:DH, :])
                    nc.sync.dma_start(k2x[1][0:DH, :], k2[DH:DH2, :])

                    # ======= phase W-B: W1/W2 transposes + AllGathers =====
                    # loads ride the sync queue; evictions+stores are batched
                    # 4-wide so the scalar queue isn't head-blocked (the v3
                    # per-chunk chain stalled enc(1..3) for ~150us)
                    with tc.tile_pool(name="stw2", bufs=2) as stw2:
                        for s in range(4):
                            w1n = stw2.tile([128, 2 * D], F32R, tag="w1n",
                                            name="w1n")
                            nc.sync.dma_start(
                                w1n[:], W1[s * 128:(s + 1) * 128, :])
                            for grp in range(4):
                                pt = pstr.tile([128, 512], F32R, tag="ps_tr",
                                               name="ps_w1t")
                                for g in range(4):
                                    jc = grp * 4 + g
                                    nc.tensor.transpose(
                                        pt[:, g * 128:(g + 1) * 128],
                                        w1n[:, jc * 128:(jc + 1) * 128],
                                        identr[:])
                                tw = stw2.tile([128, 512], BF16, tag="tw1",
                                               name="tw1")
                                nc.scalar.copy(tw[:], pt[:])
                                nc.scalar.dma_start(
                                    w1t_slice[grp * 512:(grp + 1) * 512,
                                              s * 128:(s + 1) * 128]
                                    .rearrange("(a p) c -> p a c", a=4),
                                    tw[:].rearrange("p (a c) -> p a c", a=4))
                        for ot in range(8):
                            o0 = ot * 128
                            oh = min(128, OUT - o0)
                            w2n = stw2.tile([128, ISL], F32R, tag="w2n",
                                            name="w2n")
                            nc.sync.dma_start(w2n[0:oh, :],
                                              W2[o0:o0 + oh, 0:ISL])
                            pt = pstr.tile([128, 512], F32R, tag="ps_tr",
                                           name="ps_w2t")
                            for ic in range(4):
                                nc.tensor.transpose(
                                    pt[:, ic * 128:ic * 128 + oh],
                                    w2n[0:oh, ic * 128:(ic + 1) * 128],
                                    identr[0:oh, 0:oh])
                            tw = stw2.tile([128, 512], BF16, tag="tw2",
                                           name="tw2")
                            nc.scalar.copy(tw[:], pt[:])
                            nc.scalar.dma_start(
                                w2t_slice[0:512, o0:o0 + oh]
                                .rearrange("(a p) c -> p a c", a=4),
                                tw[:].rearrange("p (a c) -> p a c",
                                                a=4)[:, :, 0:oh])
                    nc.gpsimd.collective_compute(
                        "AllGather", OP.bypass, replica_groups=rg,
                        ins=[w1t_slice.opt()], outs=[w1t_all.opt()])
                    nc.gpsimd.collective_compute(
                        "AllGather", OP.bypass, replica_groups=rg,
                        ins=[w2t_slice.opt()], outs=[w2t_all.opt()])

                newton_pair(0)

                # ===== pass B + AV + Wo + RS, pipelined with pass A =======
                with (
                    tc.tile_pool(name="psb", bufs=2, space="PSUM") as psb,
                    tc.tile_pool(name="pavp", bufs=1, space="PSUM") as pavp,
                    tc.tile_pool(name="ptp", bufs=2) as ptp,
                    tc.tile_pool(name="stb", bufs=2) as stb,
                ):
                    def pass_b2_chunk(p, mb, pva, pvb):
                        # the two query blocks of pair p share each
                        # stationary: consecutive same-lhsT matmuls let the
                        # PE skip/overlap the duplicate weight load
                        for mtl in range(4):
                            mt = mb * 4 + mtl
                            pTs = []
                            for h in range(HPC):
                                kst = k2x[h][0:KA, mt * 128:(mt + 1) * 128]
                                pab = []
                                for o in range(2):
                                    psB = psb.tile([128, 512], F32,
                                                   tag=f"b{h}",
                                                   name=f"ps_b{h}")
                                    nc.tensor.matmul(
                                        psB[:], kst,
                                        q2x[h][0:KA, (2 * p + o) * 512:
                                               (2 * p + o + 1) * 512],
                                        start=True, stop=True)
                                    pab.append(psB)
                                for o in range(2):
                                    pT = ptp.tile([128, 512], BF16,
                                                  tag=f"pt{h}{o}",
                                                  name=f"pt{h}{o}")
                                    nc.scalar.activation(pT[:], pab[o][:],
                                                         AF.Relu)
                                    pTs.append(pT)
                            for h in range(HPC):
                                for o, pv in ((0, pva), (1, pvb)):
                                    nc.tensor.matmul(
                                        pv[h * DH:(h + 1) * DH, :],
                                        v2[h][:, mt, :], pTs[2 * h + o][:],
                                        start=(mt == 0),
                                        stop=(mt == 31 and h == 1),
                                        skip_group_check=True)

                    def pass_b_tail(nb, pav):
                        pavS = stb.tile([128, 512], BF16, tag="pavs",
                                        name="pavs")
                        nc.scalar.copy(pavS[:], pav[:])
                        for nsub in range(4):
                            for dc in range(2):
                                psW = psb.tile([128, 512], F32,
                                               tag="b0", name="ps_wo2")
                                nc.tensor.matmul(
                                    psW[:],
                                    pavS[:, nsub * 128:(nsub + 1) * 128],
                                    woT[:, dc * 512:(dc + 1) * 512],
                                    start=True, stop=True)
                                so = stb.tile([128, 512], BF16, tag="so_wo",
                                              name="so_wo")
                                nc.scalar.copy(so[:], psW[:])
                                nc.sync.dma_start(
                                    proj_part[nb * 512 + nsub * 128:
                                              nb * 512 + (nsub + 1) * 128,
                                              dc * 512:(dc + 1) * 512],
                                    so[:])
                        nc.gpsimd.collective_compute(
                            "ReduceScatter", OP.add, replica_groups=rg,
                            ins=[proj_part[nb * 512:(nb + 1) * 512, :]],
                            outs=[proj_loc[nb]])

                    pv0 = pavp.tile([128, 512], F32, tag="pav0", name="pav0")
                    pv1 = pavp.tile([128, 512], F32, tag="pav1", name="pav1")
                    for mb in range(8):
                        pass_a(2, mb)
                        pass_a(3, mb)
                        pass_b2_chunk(0, mb, pv0, pv1)
                    pass_b_tail(0, pv0)
                    pass_b_tail(1, pv1)
                    newton_pair(1)
                    pv2 = pavp.tile([128, 512], F32, tag="pav0", name="pav2")
                    pv3 = pavp.tile([128, 512], F32, tag="pav1", name="pav3")
                    for mb in range(8):
                        pass_b2_chunk(1, mb, pv2, pv3)
                    pass_b_tail(2, pv2)
                    pass_b_tail(3, pv3)

            # ===================== MLP (data parallel) =====================
            with (
                tc.tile_pool(name="stm", bufs=2) as stm,
                tc.tile_pool(name="wst", bufs=6) as wst,
                tc.tile_pool(name="psm", bufs=1, space="PSUM") as psm,
                tc.tile_pool(name="psmt", bufs=2, space="PSUM") as psmt,
                tc.tile_pool(name="psm2", bufs=1, space="PSUM") as psm2,
            ):
                # proj_loc -> finT rows 1024..2047 via DMA XBAR transpose
                for j in range(4):
                    for dc in range(8):
                        nc.scalar.dma_start(
                            finT[:, 8 + dc, j * 64:(j + 1) * 64],
                            proj_loc[j][:, dc * 128:(dc + 1) * 128],
                            transpose=True)

                # MLP1: weight stream split over both HWDGE queues in
                # 256-row chunks; h^T via PE bf16 transpose + DVE eviction
                for r in range(8):
                    psH = [psm.tile([128, 512], F32, tag=f"ps_h{n2}",
                                    name=f"ps_h{n2}") for n2 in range(2)]
                    for kc2 in range(8):
                        w1s = wst.tile([128, 2, 512], BF16, tag="w1s",
                                       name="w1s")
                        dq = nc.sync if (kc2 % 2 == 0) else nc.scalar
                        dq.dma_start(
                            w1s[:],
                            w1t_all[r * 2048 + kc2 * 256:
                                    r * 2048 + (kc2 + 1) * 256, :]
                            .rearrange("(a p) c -> p a c", a=2))
                        for jj in range(2):
                            kc = kc2 * 2 + jj
                            for n2 in range(2):
                                nc.tensor.matmul(
                                    psH[n2][:],
                                    finT[:, kc, n2 * 128:(n2 + 1) * 128],
                                    w1s[:, jj, :],
                                    start=(kc == 0), stop=(kc == 15))
                    for n2 in range(2):
                        hr = stm.tile([128, 512], BF16, tag=f"hr{n2}",
                                      name="hr")
                        nc.scalar.activation(hr[:], psH[n2][:], AF.Relu)
                        for sub in range(4):
                            ptb = psmt.tile([128, 128], BF16, tag="ps_htr",
                                            name="ps_htr")
                            nc.tensor.transpose(
                                ptb[:], hr[:, sub * 128:(sub + 1) * 128],
                                identb[:])
                            nc.vector.tensor_copy(
                                hT[:, r * 4 + sub,
                                   n2 * 128:(n2 + 1) * 128], ptb[:])

                # MLP2: y = hT.T @ w2T
                for oc in range(2):
                    psY = [psm2.tile([128, OC], F32, tag=f"ps_y{n2}",
                                     name=f"ps_y{n2}") for n2 in range(2)]
                    for kd in range(16):
                        w2s = wst.tile([128, 2, OC], BF16, tag="w2s",
                                       name="w2s")
                        dq = nc.sync if (kd % 2 == 0) else nc.scalar
                        dq.dma_start(
                            w2s[:],
                            w2t_all[kd * 256:(kd + 1) * 256,
                                    oc * OC:(oc + 1) * OC]
                            .rearrange("(a p) c -> p a c", a=2))
                        for jj in range(2):
                            kc2 = kd * 2 + jj
                            for n2 in range(2):
                                nc.tensor.matmul(
                                    psY[n2][:],
                                    hT[:, kc2, n2 * 128:(n2 + 1) * 128],
                                    w2s[:, jj, :],
                                    start=(kc2 == 0), stop=(kc2 == 31))
                    for n2 in range(2):
                        yv = stm.tile([128, OC], F32, tag=f"yv{n2}",
                                      name="yv")
                        nc.scalar.copy(yv[:], psY[n2][:])
                        nc.sync.dma_start(
                            y[n2 * 128:(n2 + 1) * 128,
                              oc * OC:(oc + 1) * OC], yv[:])

        glob_ctx.__exit__(None, None, None)

    nc.compile()
    return nc


_BUILT = None


def _get_built():
    global _BUILT
    if _BUILT is None:
        _BUILT = build_kernel()
    return _BUILT


def _core_query_index(c):
    """Global query rows owned by core c (4 groups of 64, from the
    per-512-block ReduceScatter layout)."""
    return np.concatenate(
        [np.arange(512 * j + 64 * c, 512 * j + 64 * c + 64)
         for j in range(4)])


def _make_in_maps(in_map):
    """Rotate weight blocks so the single SPMD program's block-0 slices pick
    out core c's shard; add the per-core enc_local rows."""
    maps = []
    enc = in_map["encoder_output"]
    for c in range(NCORES):
        m = dict(in_map)
        m["enc_local"] = np.ascontiguousarray(enc[_core_query_index(c)])
        if c:
            m["Wq"] = np.ascontiguousarray(np.roll(in_map["Wq"], -c * DH2, 0))
            m["Wk"] = np.ascontiguousarray(np.roll(in_map["Wk"], -c * DH2, 0))
            m["Wv"] = np.ascontiguousarray(np.roll(in_map["Wv"], -c * DH2, 0))
            m["Wo"] = np.ascontiguousarray(np.roll(in_map["Wo"], -c * DH2, 1))
            m["W1"] = np.ascontiguousarray(np.roll(in_map["W1"], -c * ISL, 0))
            m["W2"] = np.ascontiguousarray(np.roll(in_map["W2"], -c * ISL, 1))
        maps.append(m)
    return maps


def _unshard_y(y_cores):
    """y_cores [NCORES, NL, OUT] -> full [N, OUT] via the RS query map."""
    out = np.empty((N, OUT), np.float32)
    for c in range(NCORES):
        out[_core_query_index(c)] = y_cores[c]
    return out


def run_on_cores(in_map, trace=False, **kw):
    nc = _get_built()
    in_maps = _make_in_maps(in_map)
    return run_bass_kernel_spmd(nc, in_maps, list(range(NCORES)),
                                trace=trace, **kw)


def kernel(**inputs) -> np.ndarray:
    names = ["encoder_output", "memory_set", "Wq", "Wk", "Wv", "Wo", "W1",
             "W2"]
    in_map = {k: np.ascontiguousarray(np.asarray(inputs[k], dtype=np.float32))
              for k in names}
    res = run_on_cores(in_map)
    return _unshard_y(np.stack([res.results[c]["y"]
                                for c in range(NCORES)])).astype(np.float32)
